# revision 1
# baseline (speedup 1.0000x reference)
"""Multi-head attention (RoPE + causal softmax + out-proj) on 8 TRN2 NeuronCores.

Sharding: core c handles batch b = c // 2 and head-half g = c % 2 (8 of 16
heads). Each core computes q/k/v projections for its heads, RoPE, causal
attention, and a partial transposed output projection
outT = (y_heads @ Wo_part.T).T; the host sums the two partials per batch.

Device layout notes:
 - All matmul operands are float32r (single-pass full-rate PE fp32).
 - q/k weight rows are permuted host-side so the RoPE even/odd pairs become
   contiguous 32-partition blocks: tiles hold [x1 of 4 heads | x2 of 4 heads]
   and RoPE runs as full-width vector ops.  Attention contracts over dh=64 as
   two K=32 matmul passes (x1, x2) per head; two heads run concurrently on
   distinct 32-row PE groups via tile_position.
 - Attention is k-major: sT = k q^T tiles [k:128, q:512]; exp on ScalarE
   (1/sqrt(dh) folded into the activation scale); causal handling is
   tile-level skipping plus a precomputed 0/1 mask multiply (on GPSIMD) for
   diagonal tiles; y^T = v_aug.T @ expT where v_aug carries a ones column
   per head, making row 64 of y^T the softmax denominator for free.
 - Normalization is per-head-pair (reciprocal chunks + K=1 PE broadcast
   matmul + DVE multiply) so it overlaps the next pair's attention.
"""

import numpy as np

B, T, C, H = 4, 2048, 1024, 16
DH = C // H  # 64
NCORES = 8
HPC = H // 2  # 8 heads per core
QR = HPC * DH  # 512 rows per q/k/v section
TS = 512  # t-chunk width
NTS = T // TS  # 4
CC = C // 128  # 8 contraction chunks
NKT = T // 128  # 16 k-tiles / t-row chunks

_CACHE = {}


def _build_program():
    import concourse.mybir as mybir
    import concourse.tile as tile
    from concourse import bacc

    f32 = mybir.dt.float32
    f32r = mybir.dt.float32r
    EXP = mybir.ActivationFunctionType.Exp

    nc = bacc.Bacc(trn_type="TRN2")

    xT = nc.dram_tensor("xT", [C, T], f32, kind="ExternalInput").ap()
    wqkvT = nc.dram_tensor("wqkvT", [C, 3 * QR], f32, kind="ExternalInput").ap()
    woT = nc.dram_tensor("woT", [QR, C], f32, kind="ExternalInput").ap()
    cosT = nc.dram_tensor("cosT", [128, T], f32, kind="ExternalInput").ap()
    sinT = nc.dram_tensor("sinT", [128, T], f32, kind="ExternalInput").ap()
    maskd = nc.dram_tensor("maskd", [128, TS + 128], f32, kind="ExternalInput").ap()
    outT = nc.dram_tensor("outT", [C, T], f32, kind="ExternalOutput").ap()

    with tile.TileContext(nc) as tc:
        with tc.tile_pool(name="persist", bufs=1) as pp:
            # rope'd q/k in projection layout: key (part, grp, half):
            # rows 32*i = x-half of local head 4*grp+i
            qk = {}
            for part in range(2):
                for grp in range(2):
                    for half in range(2):
                        nm = f"qk{part}{grp}{half}"
                        qk[(part, grp, half)] = pp.tile(
                            [128, T], f32r, tag=nm, name=nm
                        )
            # v with a ones column per head: [t-chunk 128, 8 * 65]
            v_aug = [
                pp.tile([128, HPC * 65], f32r, tag=f"va{t}", name=f"va{t}")
                for t in range(NKT)
            ]

            # ---------------- phase A: qkv projection + rope ----------------
            with (
                tc.tile_pool(name="wpool", bufs=1) as wp,
                tc.tile_pool(name="xpool", bufs=9) as xp,
                tc.tile_pool(name="trig", bufs=1) as tp,
                tc.tile_pool(name="ropetmp", bufs=6) as rt,
                tc.tile_pool(name="psA", bufs=4, space="PSUM") as psA,
            ):
                # first weight chunk, then first x chunk set, then the rest —
                # so the first matmul group isn't stuck behind 6 MB of weights
                wtiles = [None] * CC
                w = wp.tile([128, 3 * QR], f32r, tag="w0", name="w0")
                nc.sync.dma_start(w[:], wqkvT[0:128, :].bitcast(f32r))
                wtiles[0] = w
                xts0 = []
                for cc in range(CC):
                    xt = xp.tile([128, TS], f32r, tag="xts", name=f"x0{cc}")
                    nc.sync.dma_start(
                        xt[:], xT[128 * cc : 128 * (cc + 1), 0:TS].bitcast(f32r)
                    )
                    xts0.append(xt)
                for cc in range(1, CC):
                    w = wp.tile([128, 3 * QR], f32r, tag=f"w{cc}", name=f"w{cc}")
                    nc.sync.dma_start(
                        w[:], wqkvT[128 * cc : 128 * (cc + 1), :].bitcast(f32r)
                    )
                    wtiles[cc] = w
                ct = tp.tile([128, T], f32, tag="cos")
                st_ = tp.tile([128, T], f32, tag="sin")
                nc.sync.dma_start(ct[:], cosT[:])
                nc.sync.dma_start(st_[:], sinT[:])

                ones8 = tp.tile([128, HPC], f32, tag="ones8")
                nc.gpsimd.memset(ones8[:], 1.0)
                for t in range(NKT):
                    var = v_aug[t].rearrange("p (h d) -> p h d", h=HPC)
                    nc.vector.tensor_copy(var[:, :, 64:65], ones8[:].unsqueeze(2))

                for ts in range(NTS):
                    if ts == 0:
                        xts = xts0
                    else:
                        xts = []
                        for cc in range(CC):
                            xt = xp.tile([128, TS], f32r, tag="xts", name=f"x{ts}{cc}")
                            nc.sync.dma_start(
                                xt[:],
                                xT[
                                    128 * cc : 128 * (cc + 1), TS * ts : TS * (ts + 1)
                                ].bitcast(f32r),
                            )
                            xts.append(xt)

                    # q/k rows -> rope (written straight into persistent tiles)
                    for part in range(2):  # 0=q, 1=k
                        for grp in range(2):  # local heads 4*grp .. 4*grp+3
                            ptiles = []
                            for half in range(2):  # x1, x2
                                p = psA.tile(
                                    [128, TS], f32, tag="proj", name=f"p{ts}{part}{grp}{half}"
                                )
                                col0 = QR * part + 256 * grp + 128 * half
                                for cc in range(CC):
                                    nc.tensor.matmul(
                                        p[:],
                                        wtiles[cc][:, col0 : col0 + 128],
                                        xts[cc][:],
                                        start=(cc == 0),
                                        stop=(cc == CC - 1),
                                    )
                                ptiles.append(p)
                            x1p, x2p = ptiles
                            csl = ct[:, TS * ts : TS * (ts + 1)]
                            ssl = st_[:, TS * ts : TS * (ts + 1)]
                            o1 = qk[(part, grp, 0)][:, TS * ts : TS * (ts + 1)]
                            o2 = qk[(part, grp, 1)][:, TS * ts : TS * (ts + 1)]
                            t1 = rt.tile([128, TS], f32, tag="rt", name=f"t1{ts}{part}{grp}")
                            t2 = rt.tile([128, TS], f32, tag="rt", name=f"t2{ts}{part}{grp}")
                            nc.vector.tensor_mul(t1[:], x1p[:], csl)
                            nc.vector.tensor_mul(t2[:], x2p[:], ssl)
                            nc.vector.tensor_sub(o1, t1[:], t2[:])
                            t3 = rt.tile([128, TS], f32, tag="rt", name=f"t3{ts}{part}{grp}")
                            t4 = rt.tile([128, TS], f32, tag="rt", name=f"t4{ts}{part}{grp}")
                            nc.vector.tensor_mul(t3[:], x1p[:], ssl)
                            nc.vector.tensor_mul(t4[:], x2p[:], csl)
                            nc.vector.tensor_add(o2, t3[:], t4[:])

                    # v projection straight into v_aug
                    for tr4 in range(4):
                        t = 4 * ts + tr4
                        p = psA.tile([128, QR], f32, tag="proj", name=f"pv{ts}{tr4}")
                        for cc in range(CC):
                            nc.tensor.matmul(
                                p[:],
                                xts[cc][:, 128 * tr4 : 128 * (tr4 + 1)],
                                wtiles[cc][:, 2 * QR : 3 * QR],
                                start=(cc == 0),
                                stop=(cc == CC - 1),
                            )
                        var = v_aug[t].rearrange("p (h d) -> p h d", h=HPC)
                        nc.vector.tensor_copy(
                            var[:, :, 0:64],
                            p[:].rearrange("p (h d) -> p h d", h=HPC),
                        )

            # ---------------- phase B: attention ----------------
            with tc.tile_pool(name="pB", bufs=1) as pb:
                yT_all = [
                    pb.tile([128, T], f32r, tag=f"ya{j}", name=f"ya{j}")
                    for j in range(4)
                ]
                mt = pb.tile([128, TS + 128], f32r, tag="mask")
                nc.sync.dma_start(mt[:], maskd[:].bitcast(f32r))
                dn = pb.tile([128, TS], f32, tag="dn")
                rcp = pb.tile([128, TS], f32, tag="rcp")

                with (
                    tc.tile_pool(name="epool", bufs=4) as ep,
                    tc.tile_pool(name="dstage", bufs=2) as dsp,
                    tc.tile_pool(name="rstage", bufs=4) as rsp,
                    tc.tile_pool(name="bstage", bufs=4) as bsp,
                    tc.tile_pool(name="psS", bufs=1, space="PSUM") as psS,
                    tc.tile_pool(name="psY", bufs=1, space="PSUM") as psY,
                ):
                    for g4 in range(2):  # head groups of 4: heads 4*g4..4*g4+3
                        for qi in range(NTS):
                            q0 = TS * qi
                            nkt = 4 * (qi + 1)
                            yTs = [
                                psY.tile(
                                    [65, TS], f32, tag=f"yT{i}", name=f"yT{g4}_{qi}_{i}"
                                )
                                for i in range(4)
                            ]
                            for kt in range(nkt):
                                k0 = 128 * kt
                                sT = psS.tile(
                                    [128, 4 * TS], f32, tag="sT", name=f"sT{g4}_{qi}_{kt}"
                                )
                                for lh4 in range(4):
                                    rb = 32 * lh4
                                    for half in range(2):
                                        nc.tensor.matmul(
                                            sT[:, TS * lh4 : TS * (lh4 + 1)],
                                            qk[(1, g4, half)][rb : rb + 32, k0 : k0 + 128],
                                            qk[(0, g4, half)][rb : rb + 32, q0 : q0 + TS],
                                            start=(half == 0),
                                            stop=(half == 1),
                                            tile_position=(rb, 0),
                                        )
                                eT = ep.tile(
                                    [128, 4 * TS], f32r, tag="eT", name=f"eT{g4}_{qi}_{kt}"
                                )
                                nc.scalar.activation(eT[:], sT[:], EXP, scale=0.125)
                                r = kt - 4 * qi
                                if r >= 0:
                                    # causal: cols < 128*r are fully masked,
                                    # then a 128-wide triangular strip.
                                    w = 128 * (r + 1)
                                    msl = mt[:, TS - 128 * r : TS + 128]
                                    ev = eT[:].rearrange("p (s q) -> p s q", s=4)
                                    nc.gpsimd.tensor_mul(
                                        ev[:, :, 0:w],
                                        ev[:, :, 0:w],
                                        msl.unsqueeze(1).broadcast_to([128, 4, w]),
                                    )
                                for lh4 in range(4):
                                    h = 4 * g4 + lh4
                                    nc.tensor.matmul(
                                        yTs[lh4][:],
                                        v_aug[kt][:, 65 * h : 65 * h + 65],
                                        eT[:, TS * lh4 : TS * (lh4 + 1)],
                                        start=(kt == 0),
                                        stop=(kt == nkt - 1),
                                    )
                            # unnormalized copy + denominator staging
                            for lh4 in range(4):
                                h = 4 * g4 + lh4
                                j, e = h // 2, h % 2
                                ridx = 64 * g4 + 4 * lh4 + qi
                                nc.vector.tensor_copy(
                                    yT_all[j][64 * e : 64 * e + 64, q0 : q0 + TS],
                                    yTs[lh4][0:64, :],
                                )
                                dtmp = dsp.tile(
                                    [65, TS], f32, tag="dt", name=f"dt{h}_{qi}"
                                )
                                nc.vector.tensor_copy(dtmp[64:65, :], yTs[lh4][64:65, :])
                                nc.sync.dma_start(
                                    dn[ridx : ridx + 1, :], dtmp[64:65, :]
                                )

                        # per-group normalization (overlaps next group's attention)
                        r0 = 64 * g4
                        nc.vector.reciprocal(
                            rcp[r0 : r0 + 16, :], dn[r0 : r0 + 16, :]
                        )
                        for lh4 in range(4):
                            h = 4 * g4 + lh4
                            j, e = h // 2, h % 2
                            for qi in range(NTS):
                                ridx = 64 * g4 + 4 * lh4 + qi
                                q0 = TS * qi
                                rtile = rsp.tile(
                                    [1, TS], f32, tag="rr", name=f"rr{h}_{qi}"
                                )
                                nc.sync.dma_start(rtile[:], rcp[ridx : ridx + 1, :])
                                bcS = bsp.tile(
                                    [128, TS], f32, tag="bb", name=f"bb{h}_{qi}"
                                )
                                nc.gpsimd.partition_broadcast(bcS[:], rtile[:])
                                ysl = yT_all[j][64 * e : 64 * e + 64, q0 : q0 + TS]
                                nc.vector.tensor_mul(
                                    ysl, ysl, bcS[64 * e : 64 * e + 64, :]
                                )

                # ---------------- out projection ----------------
                with (
                    tc.tile_pool(name="wopool", bufs=1) as wop,
                    tc.tile_pool(name="ostage", bufs=4) as osp,
                    tc.tile_pool(name="psW", bufs=4, space="PSUM") as psW,
                ):
                    wot = []
                    for cc in range(4):
                        w = wop.tile([128, C], f32r, tag=f"wo{cc}", name=f"wo{cc}")
                        nc.sync.dma_start(
                            w[:], woT[128 * cc : 128 * (cc + 1), :].bitcast(f32r)
                        )
                        wot.append(w)
                    for ts in range(NTS):
                        for co in range(8):
                            p = psW.tile([128, TS], f32, tag="op", name=f"o{ts}{co}")
                            for cc in range(4):
                                nc.tensor.matmul(
                                    p[:],
                                    wot[cc][:, 128 * co : 128 * (co + 1)],
                                    yT_all[cc][:, TS * ts : TS * (ts + 1)],
                                    start=(cc == 0),
                                    stop=(cc == 3),
                                )
                            o = osp.tile([128, TS], f32, tag="os", name=f"os{ts}{co}")
                            nc.scalar.copy(o[:], p[:])
                            nc.sync.dma_start(
                                outT[
                                    128 * co : 128 * (co + 1), TS * ts : TS * (ts + 1)
                                ],
                                o[:],
                            )

    nc.compile()
    return nc


def _get_program():
    if "nc" not in _CACHE:
        _CACHE["nc"] = _build_program()
    return _CACHE["nc"]


def _host_inputs(x, cos, sin, Wqkv, Wo):
    """Build the 8 per-core input maps."""
    # permutation of one head-section's 512 rows (head-relative):
    # row-tile layout [x1 h0-3 | x2 h0-3 | x1 h4-7 | x2 h4-7], 32 rows/block
    perm = []
    for grp in range(2):
        for half in range(2):
            for lh in range(4 * grp, 4 * grp + 4):
                for jj in range(32):
                    perm.append(64 * lh + 2 * jj + half)
    perm = np.asarray(perm)

    cosT4 = np.ascontiguousarray(np.tile(cos.T, (4, 1)).astype(np.float32))
    sinT4 = np.ascontiguousarray(np.tile(sin.T, (4, 1)).astype(np.float32))

    # mask [128, 512+128]: 512 zero cols then a lower-triangular 128 block
    tri = (np.arange(128)[:, None] <= np.arange(128)[None, :]).astype(np.float32)
    maskd = np.ascontiguousarray(
        np.concatenate([np.zeros((128, TS), np.float32), tri], axis=1)
    )

    in_maps = []
    for c in range(NCORES):
        b, g = c // 2, c % 2
        hs0 = HPC * g
        sec = np.arange(QR) + DH * hs0  # this core's rows within a section
        Wq = Wqkv[sec[perm], :]
        Wk = Wqkv[C + sec[perm], :]
        Wv = Wqkv[2 * C + sec, :]
        wqkvT = np.ascontiguousarray(np.concatenate([Wq, Wk, Wv], 0).T)
        woTc = np.ascontiguousarray(Wo[:, sec].T)
        xTb = np.ascontiguousarray(x[b].T)
        in_maps.append(
            {
                "xT": xTb,
                "wqkvT": wqkvT,
                "woT": woTc,
                "cosT": cosT4,
                "sinT": sinT4,
                "maskd": maskd,
            }
        )
    return in_maps


def kernel(x, cos, sin, Wqkv, Wo, _want_profile=False):
    from concourse.bass_utils import run_bass_kernel_spmd

    x = np.asarray(x, dtype=np.float32)
    cos = np.asarray(cos, dtype=np.float32)
    sin = np.asarray(sin, dtype=np.float32)
    Wqkv = np.asarray(Wqkv, dtype=np.float32)
    Wo = np.asarray(Wo, dtype=np.float32)

    nc = _get_program()
    in_maps = _host_inputs(x, cos, sin, Wqkv, Wo)
    res = run_bass_kernel_spmd(nc, in_maps, list(range(NCORES)), trace=_want_profile)
    out = np.empty((B, T, C), dtype=np.float32)
    for b in range(B):
        acc = (
            res.results[2 * b]["outT"].astype(np.float32)
            + res.results[2 * b + 1]["outT"].astype(np.float32)
        )
        out[b] = acc.T
    if _want_profile:
        return out, res
    return out



# revision 5
# speedup vs baseline: 1.5310x; 1.5310x over previous
"""Multi-head attention (RoPE + causal softmax + out-proj) on 8 TRN2 NeuronCores.

Sharding: core c handles batch b = c // 2 and head-half g = c % 2 (8 of 16
heads). Each core computes q/k/v projections for its heads, RoPE, causal
attention, and a partial transposed output projection
outT = (y_heads @ Wo_part.T).T; the host sums the two partials per batch.

Device layout notes (v2 - bf16 attention pipeline):
 - Projection matmuls run fp32-accumulating over bf16 operands (x, W cast
   host-side).  PSUM results are evacuated by ScalarE (idle in this phase)
   to bf16 SBUF, and RoPE runs as bf16 DVE ops at 2x.
 - q/k weight rows are permuted host-side so RoPE even/odd pairs become
   contiguous 32-partition blocks; attention contracts dh=64 as two K=32
   passes per head on distinct 32-row PE groups.  Score matmuls are issued
   in 4-wide waves (x1 of 4 heads, then x2 of 4 heads) so the four
   row-groups run concurrently.
 - Scores live in two per-2-head PSUM tiles [128, 2*512]; exp runs on
   ScalarE writing bf16; causal diagonal blocks are q-sliced (only the
   valid 512-128r columns are computed) and a 128-wide triangle mask
   multiply on DVE handles the diagonal strip.
 - v_aug carries a ones column per head so row 64 of yT is the softmax
   denominator; normalization = DVE reciprocal + partition broadcast +
   bf16 multiplies into the persistent yT tiles.
 - Out-projection runs in bf16 with N=1024 moving tiles.
"""

import numpy as np

B, T, C, H = 4, 2048, 1024, 16
DH = C // H  # 64
NCORES = 8
HPC = H // 2  # 8 heads per core
QR = HPC * DH  # 512 rows per q/k/v section
TS = 512  # t-chunk width
NTS = T // TS  # 4
CC = C // 128  # 8 contraction chunks
NKT = T // 128  # 16 k-tiles / t-row chunks

_CACHE = {}


def _build_program():
    import concourse.mybir as mybir
    import concourse.tile as tile
    from concourse import bacc

    f32 = mybir.dt.float32
    bf16 = mybir.dt.bfloat16
    EXP = mybir.ActivationFunctionType.Exp

    nc = bacc.Bacc(trn_type="TRN2")

    xT = nc.dram_tensor("xT", [C, T], bf16, kind="ExternalInput").ap()
    wqkvT = nc.dram_tensor("wqkvT", [C, 3 * QR], bf16, kind="ExternalInput").ap()
    woT = nc.dram_tensor("woT", [QR, C], bf16, kind="ExternalInput").ap()
    cosT = nc.dram_tensor("cosT", [128, T], bf16, kind="ExternalInput").ap()
    sinT = nc.dram_tensor("sinT", [128, T], bf16, kind="ExternalInput").ap()
    trid = nc.dram_tensor("trid", [128, 128], bf16, kind="ExternalInput").ap()
    outT = nc.dram_tensor("outT", [C, T], f32, kind="ExternalOutput").ap()

    with tile.TileContext(nc) as tc:
        with tc.tile_pool(name="persist", bufs=1) as pp:
            # rope'd q/k in projection layout: key (part, grp, half):
            # rows 32*i = x-half of local head 4*grp+i
            qk = {}
            for part in range(2):
                for grp in range(2):
                    for half in range(2):
                        nm = f"qk{part}{grp}{half}"
                        qk[(part, grp, half)] = pp.tile(
                            [128, T], bf16, tag=nm, name=nm
                        )
            # v with a ones column per head: [t-chunk 128, 8 * 65]
            v_aug = [
                pp.tile([128, HPC * 65], bf16, tag=f"va{t}", name=f"va{t}")
                for t in range(NKT)
            ]
            yT_all = [
                pp.tile([128, T], bf16, tag=f"ya{j}", name=f"ya{j}")
                for j in range(4)
            ]

            # ---------------- phase A: qkv projection + rope ----------------
            with (
                tc.tile_pool(name="wpool", bufs=1) as wp,
                tc.tile_pool(name="xpool", bufs=9) as xp,
                tc.tile_pool(name="trig", bufs=1) as tp,
                tc.tile_pool(name="evac", bufs=4) as evp,
                tc.tile_pool(name="ropetmp", bufs=6) as rt,
                tc.tile_pool(name="psA", bufs=6, space="PSUM") as psA,
            ):
                # first weight chunk, then first x chunk set, then the rest —
                # so the first matmul group isn't stuck behind the weights
                wtiles = [None] * CC
                w = wp.tile([128, 3 * QR], bf16, tag="w0", name="w0")
                nc.sync.dma_start(w[:], wqkvT[0:128, :])
                wtiles[0] = w
                xts0 = []
                for cc in range(CC):
                    xt = xp.tile([128, TS], bf16, tag="xts", name=f"x0{cc}")
                    nc.sync.dma_start(xt[:], xT[128 * cc : 128 * (cc + 1), 0:TS])
                    xts0.append(xt)
                for cc in range(1, CC):
                    w = wp.tile([128, 3 * QR], bf16, tag=f"w{cc}", name=f"w{cc}")
                    nc.sync.dma_start(
                        w[:], wqkvT[128 * cc : 128 * (cc + 1), :]
                    )
                    wtiles[cc] = w
                ct = tp.tile([128, T], bf16, tag="cos")
                st_ = tp.tile([128, T], bf16, tag="sin")
                nc.sync.dma_start(ct[:], cosT[:])
                nc.sync.dma_start(st_[:], sinT[:])

                ones8 = tp.tile([128, HPC], bf16, tag="ones8")
                nc.gpsimd.memset(ones8[:], 1.0)
                for t in range(NKT):
                    var = v_aug[t].rearrange("p (h d) -> p h d", h=HPC)
                    nc.vector.tensor_copy(var[:, :, 64:65], ones8[:].unsqueeze(2))

                for ts in range(NTS):
                    if ts == 0:
                        xts = xts0
                    else:
                        xts = []
                        for cc in range(CC):
                            xt = xp.tile([128, TS], bf16, tag="xts", name=f"x{ts}{cc}")
                            nc.sync.dma_start(
                                xt[:],
                                xT[
                                    128 * cc : 128 * (cc + 1), TS * ts : TS * (ts + 1)
                                ],
                            )
                            xts.append(xt)

                    # q/k rows -> rope (evac by ScalarE, rope in bf16)
                    for part in range(2):  # 0=q, 1=k
                        for grp in range(2):  # local heads 4*grp .. 4*grp+3
                            stiles = []
                            for half in range(2):  # x1, x2
                                p = psA.tile(
                                    [128, TS], f32, tag="proj", name=f"p{ts}{part}{grp}{half}"
                                )
                                col0 = QR * part + 256 * grp + 128 * half
                                for cc in range(CC):
                                    nc.tensor.matmul(
                                        p[:],
                                        wtiles[cc][:, col0 : col0 + 128],
                                        xts[cc][:],
                                        start=(cc == 0),
                                        stop=(cc == CC - 1),
                                    )
                                s = evp.tile(
                                    [128, TS], bf16, tag="ev", name=f"e{ts}{part}{grp}{half}"
                                )
                                nc.scalar.copy(s[:], p[:])
                                stiles.append(s)
                            x1s, x2s = stiles
                            csl = ct[:, TS * ts : TS * (ts + 1)]
                            ssl = st_[:, TS * ts : TS * (ts + 1)]
                            o1 = qk[(part, grp, 0)][:, TS * ts : TS * (ts + 1)]
                            o2 = qk[(part, grp, 1)][:, TS * ts : TS * (ts + 1)]
                            t1 = rt.tile([128, TS], bf16, tag="rt", name=f"t1{ts}{part}{grp}")
                            t2 = rt.tile([128, TS], bf16, tag="rt", name=f"t2{ts}{part}{grp}")
                            nc.vector.tensor_mul(t1[:], x1s[:], csl)
                            nc.vector.tensor_mul(t2[:], x2s[:], ssl)
                            nc.vector.tensor_sub(o1, t1[:], t2[:])
                            t3 = rt.tile([128, TS], bf16, tag="rt", name=f"t3{ts}{part}{grp}")
                            t4 = rt.tile([128, TS], bf16, tag="rt", name=f"t4{ts}{part}{grp}")
                            nc.vector.tensor_mul(t3[:], x1s[:], ssl)
                            nc.vector.tensor_mul(t4[:], x2s[:], csl)
                            nc.vector.tensor_add(o2, t3[:], t4[:])

                    # v projection straight into v_aug
                    for tr4 in range(4):
                        t = 4 * ts + tr4
                        p = psA.tile([128, QR], f32, tag="proj", name=f"pv{ts}{tr4}")
                        for cc in range(CC):
                            nc.tensor.matmul(
                                p[:],
                                xts[cc][:, 128 * tr4 : 128 * (tr4 + 1)],
                                wtiles[cc][:, 2 * QR : 3 * QR],
                                start=(cc == 0),
                                stop=(cc == CC - 1),
                            )
                        var = v_aug[t].rearrange("p (h d) -> p h d", h=HPC)
                        nc.vector.tensor_copy(
                            var[:, :, 0:64],
                            p[:].rearrange("p (h d) -> p h d", h=HPC),
                        )

            # ---------------- phase B: attention ----------------
            with tc.tile_pool(name="pB", bufs=1) as pb:
                trit = pb.tile([128, 128], bf16, tag="tri")
                nc.sync.dma_start(trit[:], trid[:])
                dn = pb.tile([128, TS], f32, tag="dn")
                rcp = pb.tile([128, TS], bf16, tag="rcp")

                with (
                    tc.tile_pool(name="epool", bufs=3) as ep,
                    tc.tile_pool(name="dstage", bufs=2) as dsp,
                    tc.tile_pool(name="rstage", bufs=4) as rsp,
                    tc.tile_pool(name="bstage", bufs=4) as bsp,
                    tc.tile_pool(name="psS", bufs=1, space="PSUM") as psS,
                    tc.tile_pool(name="psY", bufs=1, space="PSUM") as psY,
                ):
                    for g4 in range(2):  # head groups of 4: heads 4*g4..4*g4+3
                        for qi in range(NTS):
                            q0 = TS * qi
                            nkt = 4 * (qi + 1)
                            yTs = [
                                psY.tile(
                                    [65, TS], f32, tag=f"yT{i}", name=f"yT{g4}_{qi}_{i}"
                                )
                                for i in range(4)
                            ]
                            for kt in range(nkt):
                                k0 = 128 * kt
                                r = kt - 4 * qi
                                qlo = 128 * r if r > 0 else 0
                                w = TS - qlo  # valid q-width of this block
                                sT2 = [
                                    psS.tile(
                                        [128, 2 * TS], f32, tag=f"sT{i}",
                                        name=f"sT{g4}_{qi}_{kt}_{i}",
                                    )
                                    for i in range(2)
                                ]
                                # score waves: x1 of all 4 heads, then x2
                                for half in range(2):
                                    for lh4 in range(4):
                                        rb = 32 * lh4
                                        col = TS * (lh4 % 2) + qlo
                                        nc.tensor.matmul(
                                            sT2[lh4 // 2][:, col : col + w],
                                            qk[(1, g4, half)][rb : rb + 32, k0 : k0 + 128],
                                            qk[(0, g4, half)][rb : rb + 32, q0 + qlo : q0 + TS],
                                            start=(half == 0),
                                            stop=(half == 1),
                                            tile_position=(rb, 0),
                                        )
                                eT2 = [
                                    ep.tile(
                                        [128, 2 * TS], bf16, tag=f"eT{i}",
                                        name=f"eT{g4}_{qi}_{kt}_{i}",
                                    )
                                    for i in range(2)
                                ]
                                for i in range(2):
                                    if qlo:
                                        sv = sT2[i].rearrange("p (s q) -> p s q", s=2)
                                        evv = eT2[i].rearrange("p (s q) -> p s q", s=2)
                                        nc.scalar.activation(
                                            evv[:, :, qlo:TS], sv[:, :, qlo:TS],
                                            EXP, scale=0.125,
                                        )
                                    else:
                                        nc.scalar.activation(
                                            eT2[i][:], sT2[i][:], EXP, scale=0.125
                                        )
                                if r >= 0:
                                    # causal: multiply the 128-wide diagonal
                                    # strip by the lower-triangular mask
                                    for i in range(2):
                                        evv = eT2[i].rearrange("p (s q) -> p s q", s=2)
                                        nc.vector.tensor_mul(
                                            evv[:, :, qlo : qlo + 128],
                                            evv[:, :, qlo : qlo + 128],
                                            trit[:].unsqueeze(1).broadcast_to(
                                                [128, 2, 128]
                                            ),
                                        )
                                for lh4 in range(4):
                                    h = 4 * g4 + lh4
                                    col = TS * (lh4 % 2) + qlo
                                    nc.tensor.matmul(
                                        yTs[lh4][:, qlo:TS],
                                        v_aug[kt][:, 65 * h : 65 * h + 65],
                                        eT2[lh4 // 2][:, col : col + w],
                                        start=(kt == 0),
                                        stop=(kt == nkt - 1),
                                    )
                            # unnormalized copy + denominator staging
                            for lh4 in range(4):
                                h = 4 * g4 + lh4
                                j, e = h // 2, h % 2
                                ridx = 64 * g4 + 4 * lh4 + qi
                                nc.vector.tensor_copy(
                                    yT_all[j][64 * e : 64 * e + 64, q0 : q0 + TS],
                                    yTs[lh4][0:64, :],
                                )
                                dtmp = dsp.tile(
                                    [65, TS], f32, tag="dt", name=f"dt{h}_{qi}"
                                )
                                nc.vector.tensor_copy(dtmp[64:65, :], yTs[lh4][64:65, :])
                                nc.sync.dma_start(
                                    dn[ridx : ridx + 1, :], dtmp[64:65, :]
                                )

                        # per-group normalization (overlaps next group's attention)
                        r0 = 64 * g4
                        with nc.allow_low_precision(
                            reason="bf16 softmax reciprocal is within tolerance"
                        ):
                            nc.vector.reciprocal(
                                rcp[r0 : r0 + 16, :], dn[r0 : r0 + 16, :]
                            )
                        for lh4 in range(4):
                            h = 4 * g4 + lh4
                            j, e = h // 2, h % 2
                            for qi in range(NTS):
                                ridx = 64 * g4 + 4 * lh4 + qi
                                q0 = TS * qi
                                rtile = rsp.tile(
                                    [1, TS], bf16, tag="rr", name=f"rr{h}_{qi}"
                                )
                                nc.sync.dma_start(rtile[:], rcp[ridx : ridx + 1, :])
                                bcS = bsp.tile(
                                    [128, TS], bf16, tag="bb", name=f"bb{h}_{qi}"
                                )
                                nc.gpsimd.partition_broadcast(bcS[:], rtile[:])
                                ysl = yT_all[j][64 * e : 64 * e + 64, q0 : q0 + TS]
                                nc.vector.tensor_mul(
                                    ysl, ysl, bcS[64 * e : 64 * e + 64, :]
                                )

                # ---------------- out projection ----------------
                with (
                    tc.tile_pool(name="wopool", bufs=1) as wop,
                    tc.tile_pool(name="ostage", bufs=4) as osp,
                    tc.tile_pool(name="psW", bufs=4, space="PSUM") as psW,
                ):
                    wot = []
                    for cc in range(4):
                        w = wop.tile([128, C], bf16, tag=f"wo{cc}", name=f"wo{cc}")
                        nc.sync.dma_start(
                            w[:], woT[128 * cc : 128 * (cc + 1), :]
                        )
                        wot.append(w)
                    for ts in range(NTS):
                        for co in range(8):
                            p = psW.tile([128, TS], f32, tag="op", name=f"o{ts}{co}")
                            for cc in range(4):
                                nc.tensor.matmul(
                                    p[:],
                                    wot[cc][:, 128 * co : 128 * (co + 1)],
                                    yT_all[cc][:, TS * ts : TS * (ts + 1)],
                                    start=(cc == 0),
                                    stop=(cc == 3),
                                )
                            o = osp.tile([128, TS], f32, tag="os", name=f"os{ts}{co}")
                            nc.scalar.copy(o[:], p[:])
                            nc.sync.dma_start(
                                outT[
                                    128 * co : 128 * (co + 1), TS * ts : TS * (ts + 1)
                                ],
                                o[:],
                            )

    nc.compile()
    return nc


def _get_program():
    if "nc" not in _CACHE:
        _CACHE["nc"] = _build_program()
    return _CACHE["nc"]


def _host_inputs(x, cos, sin, Wqkv, Wo):
    """Build the 8 per-core input maps."""
    import ml_dtypes

    bf16 = ml_dtypes.bfloat16
    # permutation of one head-section's 512 rows (head-relative):
    # row-tile layout [x1 h0-3 | x2 h0-3 | x1 h4-7 | x2 h4-7], 32 rows/block
    perm = []
    for grp in range(2):
        for half in range(2):
            for lh in range(4 * grp, 4 * grp + 4):
                for jj in range(32):
                    perm.append(64 * lh + 2 * jj + half)
    perm = np.asarray(perm)

    cosT4 = np.ascontiguousarray(np.tile(cos.T, (4, 1)).astype(bf16))
    sinT4 = np.ascontiguousarray(np.tile(sin.T, (4, 1)).astype(bf16))

    # lower-triangular (k <= q) diagonal-strip mask
    tri = (np.arange(128)[:, None] <= np.arange(128)[None, :]).astype(bf16)
    tri = np.ascontiguousarray(tri)

    in_maps = []
    for c in range(NCORES):
        b, g = c // 2, c % 2
        hs0 = HPC * g
        sec = np.arange(QR) + DH * hs0  # this core's rows within a section
        Wq = Wqkv[sec[perm], :]
        Wk = Wqkv[C + sec[perm], :]
        Wv = Wqkv[2 * C + sec, :]
        wqkvT = np.ascontiguousarray(np.concatenate([Wq, Wk, Wv], 0).T.astype(bf16))
        woTc = np.ascontiguousarray(Wo[:, sec].T.astype(bf16))
        xTb = np.ascontiguousarray(x[b].T.astype(bf16))
        in_maps.append(
            {
                "xT": xTb,
                "wqkvT": wqkvT,
                "woT": woTc,
                "cosT": cosT4,
                "sinT": sinT4,
                "trid": tri,
            }
        )
    return in_maps


def kernel(x, cos, sin, Wqkv, Wo, _want_profile=False):
    from concourse.bass_utils import run_bass_kernel_spmd

    x = np.asarray(x, dtype=np.float32)
    cos = np.asarray(cos, dtype=np.float32)
    sin = np.asarray(sin, dtype=np.float32)
    Wqkv = np.asarray(Wqkv, dtype=np.float32)
    Wo = np.asarray(Wo, dtype=np.float32)

    nc = _get_program()
    in_maps = _host_inputs(x, cos, sin, Wqkv, Wo)
    res = run_bass_kernel_spmd(nc, in_maps, list(range(NCORES)), trace=_want_profile)
    out = np.empty((B, T, C), dtype=np.float32)
    for b in range(B):
        acc = (
            res.results[2 * b]["outT"].astype(np.float32)
            + res.results[2 * b + 1]["outT"].astype(np.float32)
        )
        out[b] = acc.T
    if _want_profile:
        return out, res
    return out


# revision 8
# speedup vs baseline: 1.9672x; 1.2849x over previous
"""Multi-head attention (RoPE + causal softmax + out-proj) on 8 TRN2 NeuronCores.

Sharding: core c handles batch b = c // 2 and head-half g = c % 2 (8 of 16
heads). Each core computes q/k/v projections for its heads, RoPE, causal
attention, and a partial transposed output projection
outT = (y_heads @ Wo_part.T).T; the host sums the two partials per batch.

v3 design - software-pipelined single PE stream:
 - All matmuls in bf16 (operands cast host-side), fp32 PSUM accumulation.
 - One unified 8-bank PSUM budget: 4 banks hold the per-(g4,qi) attention
   accumulators yTs (M=65: v plus a ones column per head -> row 64 is the
   softmax denominator), 4 banks form a rotating slot pool shared by score
   tiles, projection tiles and out-projection tiles.
 - Scores: per-head [128,512] slot, two 4-wide matmul waves (x1 of 4 heads
   on distinct 32-row PE groups, then x2) -> row-group concurrency.
 - exp on ScalarE (bf16 out); causal diagonal blocks compute only the
   valid q-slice; a 128-wide triangle mask multiply runs on DVE.
 - av matmuls are emitted ONE BLOCK LATE so the in-order PE queue never
   waits on exp: while block n's exp runs, the PE executes block n+1's
   score waves and interleaved projection work.
 - Phase-A projection groups (ts=qi+1) and out-projection chunks
   (ts=qi-1) are interleaved between attention blocks as PE filler, which
   also keeps the PE HAM clock un-throttled.
 - Normalization per (g4,qi): denominator rows staged to SBUF, fast
   approximate reciprocal, GPSIMD partition broadcast, bf16 multiply.
"""

import numpy as np

B, T, C, H = 4, 2048, 1024, 16
DH = C // H  # 64
NCORES = 8
HPC = H // 2  # 8 heads per core
QR = HPC * DH  # 512 rows per q/k/v section
TS = 512  # t-chunk width
NTS = T // TS  # 4
CC = C // 128  # 8 contraction chunks
NKT = T // 128  # 16 k-tiles / t-row chunks

_CACHE = {}


def _build_program():
    import concourse.mybir as mybir
    import concourse.tile as tile
    from concourse import bacc

    f32 = mybir.dt.float32
    bf16 = mybir.dt.bfloat16
    EXP = mybir.ActivationFunctionType.Exp

    nc = bacc.Bacc(trn_type="TRN2")

    xT = nc.dram_tensor("xT", [C, T], bf16, kind="ExternalInput").ap()
    wqkvT = nc.dram_tensor("wqkvT", [C, 3 * QR], bf16, kind="ExternalInput").ap()
    woT = nc.dram_tensor("woT", [QR, C], bf16, kind="ExternalInput").ap()
    cosT = nc.dram_tensor("cosT", [128, T], bf16, kind="ExternalInput").ap()
    sinT = nc.dram_tensor("sinT", [128, T], bf16, kind="ExternalInput").ap()
    trid = nc.dram_tensor("trid", [128, 128], bf16, kind="ExternalInput").ap()
    outT = nc.dram_tensor("outT", [C, T], f32, kind="ExternalOutput").ap()

    with tile.TileContext(nc) as tc:
        with (
            tc.tile_pool(name="persist", bufs=1) as pp,
            tc.tile_pool(name="wpool", bufs=1) as wp,
            tc.tile_pool(name="xpool", bufs=9) as xp,
            tc.tile_pool(name="evac", bufs=6) as evp,
            tc.tile_pool(name="ropetmp", bufs=6) as rt,
            tc.tile_pool(name="epool", bufs=10) as ep,
            tc.tile_pool(name="dstage", bufs=3) as dsp,
            tc.tile_pool(name="rstage", bufs=4) as rsp,
            tc.tile_pool(name="bstage", bufs=4) as bsp,
            tc.tile_pool(name="ostage", bufs=4) as osp,
            tc.tile_pool(name="psB", bufs=1, space="PSUM") as psB,
        ):
            # ---------------- persistent SBUF ----------------
            qk = {}
            for part in range(2):
                for grp in range(2):
                    for half in range(2):
                        nm = f"qk{part}{grp}{half}"
                        qk[(part, grp, half)] = pp.tile([128, T], bf16, tag=nm, name=nm)
            v_aug = [
                pp.tile([128, HPC * 65], bf16, tag=f"va{t}", name=f"va{t}")
                for t in range(NKT)
            ]
            yT_all = [pp.tile([128, T], bf16, tag=f"ya{j}", name=f"ya{j}") for j in range(4)]
            ct = pp.tile([128, T], bf16, tag="cos")
            st_ = pp.tile([128, T], bf16, tag="sin")
            trit = pp.tile([128, 128], bf16, tag="tri")
            dn = pp.tile([128, TS], f32, tag="dn")
            rcpb = pp.tile([128, TS], bf16, tag="rcpb")
            ones8 = pp.tile([128, HPC], bf16, tag="ones8")
            dume = pp.tile([1, 16], bf16, tag="dume")
            dums = pp.tile([1, 16], f32, tag="dums")

            # ---------------- input DMAs ----------------
            wtiles = [None] * CC
            w = wp.tile([128, 3 * QR], bf16, tag="w0", name="w0")
            nc.sync.dma_start(w[:], wqkvT[0:128, :])
            wtiles[0] = w
            xts_all = {}
            for cc in range(CC):
                xt = xp.tile([128, TS], bf16, tag="xts", name=f"x0{cc}")
                nc.sync.dma_start(xt[:], xT[128 * cc : 128 * (cc + 1), 0:TS])
                xts_all[(0, cc)] = xt
            for cc in range(1, CC):
                w = wp.tile([128, 3 * QR], bf16, tag=f"w{cc}", name=f"w{cc}")
                nc.sync.dma_start(w[:], wqkvT[128 * cc : 128 * (cc + 1), :])
                wtiles[cc] = w
            nc.sync.dma_start(ct[:], cosT[:])
            nc.sync.dma_start(st_[:], sinT[:])
            nc.sync.dma_start(trit[:], trid[:])
            wot = []
            for cc in range(4):
                w = wp.tile([128, C], bf16, tag=f"wo{cc}", name=f"wo{cc}")
                nc.sync.dma_start(w[:], woT[128 * cc : 128 * (cc + 1), :])
                wot.append(w)

            nc.gpsimd.memset(ones8[:], 1.0)
            nc.gpsimd.memset(dums[:], 0.0)
            # preload the exp table set before the attention stream needs it
            nc.scalar.activation(dume[:], dums[:], EXP, scale=1.0)
            for t in range(NKT):
                var = v_aug[t].rearrange("p (h d) -> p h d", h=HPC)
                nc.vector.tensor_copy(var[:, :, 64:65], ones8[:].unsqueeze(2))

            # ---------------- rotating PSUM slot pool ----------------
            slot_state = {"i": 0, "n": 0}

            def slot_tile(kind):
                tag = f"s{slot_state['i'] % 4}"
                slot_state["i"] += 1
                slot_state["n"] += 1
                return psB.tile([128, TS], f32, tag=tag, name=f"{kind}{slot_state['n']}")

            # ---------------- emitters ----------------
            def emit_x_dma(ts):
                for cc in range(CC):
                    xt = xp.tile([128, TS], bf16, tag="xts", name=f"x{ts}{cc}")
                    nc.sync.dma_start(
                        xt[:], xT[128 * cc : 128 * (cc + 1), TS * ts : TS * (ts + 1)]
                    )
                    xts_all[(ts, cc)] = xt

            rope_half = {}

            def emit_aqk_half(ts, part, grp, half):
                """One projection half-group: 8 matmuls into one slot, evac.
                On half==1 also emits the rope ops for the (part, grp) pair."""
                p = slot_tile("ap")
                col0 = QR * part + 256 * grp + 128 * half
                for cc in range(CC):
                    nc.tensor.matmul(
                        p[:],
                        wtiles[cc][:, col0 : col0 + 128],
                        xts_all[(ts, cc)][:],
                        start=(cc == 0),
                        stop=(cc == CC - 1),
                    )
                s = evp.tile([128, TS], bf16, tag="ev", name=f"e{ts}{part}{grp}{half}")
                nc.vector.tensor_copy(s[:], p[:])
                rope_half[(ts, part, grp, half)] = s
                if half == 1:
                    x1s = rope_half.pop((ts, part, grp, 0))
                    x2s = s
                    csl = ct[:, TS * ts : TS * (ts + 1)]
                    ssl = st_[:, TS * ts : TS * (ts + 1)]
                    o1 = qk[(part, grp, 0)][:, TS * ts : TS * (ts + 1)]
                    o2 = qk[(part, grp, 1)][:, TS * ts : TS * (ts + 1)]
                    t1 = rt.tile([128, TS], bf16, tag="rt", name=f"t1{ts}{part}{grp}")
                    t2 = rt.tile([128, TS], bf16, tag="rt", name=f"t2{ts}{part}{grp}")
                    nc.vector.tensor_mul(t1[:], x1s[:], csl)
                    nc.vector.tensor_mul(t2[:], x2s[:], ssl)
                    nc.vector.tensor_sub(o1, t1[:], t2[:])
                    t3 = rt.tile([128, TS], bf16, tag="rt", name=f"t3{ts}{part}{grp}")
                    t4 = rt.tile([128, TS], bf16, tag="rt", name=f"t4{ts}{part}{grp}")
                    nc.vector.tensor_mul(t3[:], x1s[:], ssl)
                    nc.vector.tensor_mul(t4[:], x2s[:], csl)
                    nc.vector.tensor_add(o2, t3[:], t4[:])

            def emit_av_proj(ts, tr4):
                """One v projection group: 8 matmuls into one slot, evac."""
                t = 4 * ts + tr4
                p = slot_tile("vp")
                for cc in range(CC):
                    nc.tensor.matmul(
                        p[:],
                        xts_all[(ts, cc)][:, 128 * tr4 : 128 * (tr4 + 1)],
                        wtiles[cc][:, 2 * QR : 3 * QR],
                        start=(cc == 0),
                        stop=(cc == CC - 1),
                    )
                var = v_aug[t].rearrange("p (h d) -> p h d", h=HPC)
                nc.vector.tensor_copy(
                    var[:, :, 0:64], p[:].rearrange("p (h d) -> p h d", h=HPC)
                )

            def a_groups(ts):
                gs = []
                for part in range(2):
                    for grp in range(2):
                        for half in range(2):
                            gs.append(lambda ts=ts, p=part, g=grp, h=half: emit_aqk_half(ts, p, g, h))
                for tr4 in range(4):
                    gs.append(lambda ts=ts, t4=tr4: emit_av_proj(ts, t4))
                return gs

            def emit_out_chunk(ts, co, evac_eng):
                p = slot_tile("op")
                for cc in range(4):
                    nc.tensor.matmul(
                        p[:],
                        wot[cc][:, 128 * co : 128 * (co + 1)],
                        yT_all[cc][:, TS * ts : TS * (ts + 1)],
                        start=(cc == 0),
                        stop=(cc == 3),
                    )
                o = osp.tile([128, TS], f32, tag="os", name=f"os{ts}{co}")
                if evac_eng == "scalar":
                    nc.scalar.copy(o[:], p[:])
                else:
                    nc.vector.tensor_copy(o[:], p[:])
                nc.sync.dma_start(
                    outT[128 * co : 128 * (co + 1), TS * ts : TS * (ts + 1)], o[:]
                )

            def o_groups(ts):
                return [
                    lambda ts=ts, c=co: emit_out_chunk(
                        ts, c, "scalar" if c % 2 else "vector"
                    )
                    for co in range(8)
                ]

            # attention state
            att = {"yTs": None, "pend": None}

            def flush_av():
                if att["pend"] is None:
                    return
                g4, qi, kt, qlo, eTs = att["pend"]
                att["pend"] = None
                nkt = 4 * (qi + 1)
                for lh4 in range(4):
                    h = 4 * g4 + lh4
                    nc.tensor.matmul(
                        att["yTs"][lh4][:, qlo:TS],
                        v_aug[kt][:, 65 * h : 65 * h + 65],
                        eTs[lh4][:, qlo:TS],
                        start=(kt == 0),
                        stop=(kt == nkt - 1),
                    )

            def emit_block(g4, qi, kt):
                q0 = TS * qi
                k0 = 128 * kt
                r = kt - 4 * qi
                qlo = 128 * r if r > 0 else 0
                if kt == 0:
                    att["yTs"] = [
                        psB.tile([65, TS], f32, tag=f"y{i}", name=f"yT{g4}_{qi}_{i}")
                        for i in range(4)
                    ]
                sTs = [slot_tile("sc") for _ in range(4)]
                for half in range(2):
                    for lh4 in range(4):
                        rb = 32 * lh4
                        nc.tensor.matmul(
                            sTs[lh4][:, qlo:TS],
                            qk[(1, g4, half)][rb : rb + 32, k0 : k0 + 128],
                            qk[(0, g4, half)][rb : rb + 32, q0 + qlo : q0 + TS],
                            start=(half == 0),
                            stop=(half == 1),
                            tile_position=(rb, 0),
                        )
                # previous block's av matmuls go behind this block's waves
                flush_av()
                eTs = [
                    ep.tile([128, TS], bf16, tag="eT", name=f"eT{g4}_{qi}_{kt}_{i}")
                    for i in range(4)
                ]
                for i in range(4):
                    nc.scalar.activation(
                        eTs[i][:, qlo:TS], sTs[i][:, qlo:TS], EXP, scale=0.125
                    )
                if r >= 0:
                    for i in range(4):
                        nc.vector.tensor_mul(
                            eTs[i][:, qlo : qlo + 128],
                            eTs[i][:, qlo : qlo + 128],
                            trit[:],
                        )
                att["pend"] = (g4, qi, kt, qlo, eTs)

            def emit_post(g4, qi):
                """Flush last av, stage copies/denominators, normalize."""
                flush_av()
                q0 = TS * qi
                yTs = att["yTs"]
                den4 = dsp.tile([4, TS], f32, tag="d4", name=f"d4{g4}{qi}")
                for lh4 in range(4):
                    h = 4 * g4 + lh4
                    j, e = h // 2, h % 2
                    nc.vector.tensor_copy(
                        yT_all[j][64 * e : 64 * e + 64, q0 : q0 + TS], yTs[lh4][0:64, :]
                    )
                    dtmp = dsp.tile([65, TS], f32, tag="dt", name=f"dt{g4}{qi}{lh4}")
                    nc.vector.tensor_copy(dtmp[64:65, :], yTs[lh4][64:65, :])
                    nc.sync.dma_start(den4[lh4 : lh4 + 1, :], dtmp[64:65, :])
                rf = rsp.tile([4, TS], f32, tag="rf", name=f"rf{g4}{qi}")
                rc4 = rsp.tile([4, TS], bf16, tag="rc", name=f"rc{g4}{qi}")
                nc.vector.reciprocal_approx_fast(rf[:], den4[:])
                with nc.allow_low_precision(reason="bf16 softmax denominators"):
                    nc.vector.tensor_copy(rc4[:], rf[:])
                for lh4 in range(4):
                    h = 4 * g4 + lh4
                    j, e = h // 2, h % 2
                    rtile = rsp.tile([1, TS], bf16, tag="rr", name=f"rr{g4}{qi}{lh4}")
                    nc.sync.dma_start(rtile[:], rc4[lh4 : lh4 + 1, :])
                    bcS = bsp.tile([128, TS], bf16, tag="bb", name=f"bb{g4}{qi}{lh4}")
                    nc.gpsimd.partition_broadcast(bcS[:], rtile[:])
                    ysl = yT_all[j][64 * e : 64 * e + 64, q0 : q0 + TS]
                    nc.vector.tensor_mul(ysl, ysl, bcS[64 * e : 64 * e + 64, :])

            # ---------------- emission schedule ----------------
            for g in a_groups(0):
                g()

            for qi in range(NTS):
                items = []
                for g4 in range(2):
                    for kt in range(4 * (qi + 1)):
                        items.append(lambda g4=g4, qi=qi, kt=kt: emit_block(g4, qi, kt))
                    items.append(lambda g4=g4, qi=qi: emit_post(g4, qi))
                fillers = []
                if qi < NTS - 1:
                    emit_x_dma(qi + 1)
                    fillers.extend(a_groups(qi + 1))
                if qi >= 1:
                    fillers.extend(o_groups(qi - 1))
                # distribute fillers evenly between attention items
                n_i, n_f = len(items), len(fillers)
                out_seq = []
                fi = 0
                for k, it in enumerate(items):
                    out_seq.append(it)
                    want = ((k + 1) * n_f) // n_i
                    while fi < want:
                        out_seq.append(fillers[fi])
                        fi += 1
                while fi < n_f:
                    out_seq.append(fillers[fi])
                    fi += 1
                for it in out_seq:
                    it()

            for g in o_groups(NTS - 1):
                g()

    nc.compile()
    return nc


def _get_program():
    if "nc" not in _CACHE:
        _CACHE["nc"] = _build_program()
    return _CACHE["nc"]


def _host_inputs(x, cos, sin, Wqkv, Wo):
    """Build the 8 per-core input maps."""
    import ml_dtypes

    bf16 = ml_dtypes.bfloat16
    # permutation of one head-section's 512 rows (head-relative):
    # row-tile layout [x1 h0-3 | x2 h0-3 | x1 h4-7 | x2 h4-7], 32 rows/block
    perm = []
    for grp in range(2):
        for half in range(2):
            for lh in range(4 * grp, 4 * grp + 4):
                for jj in range(32):
                    perm.append(64 * lh + 2 * jj + half)
    perm = np.asarray(perm)

    cosT4 = np.ascontiguousarray(np.tile(cos.T, (4, 1)).astype(bf16))
    sinT4 = np.ascontiguousarray(np.tile(sin.T, (4, 1)).astype(bf16))

    # lower-triangular (k <= q) diagonal-strip mask
    tri = (np.arange(128)[:, None] <= np.arange(128)[None, :]).astype(bf16)
    tri = np.ascontiguousarray(tri)

    in_maps = []
    for c in range(NCORES):
        b, g = c // 2, c % 2
        hs0 = HPC * g
        sec = np.arange(QR) + DH * hs0  # this core's rows within a section
        Wq = Wqkv[sec[perm], :]
        Wk = Wqkv[C + sec[perm], :]
        Wv = Wqkv[2 * C + sec, :]
        wqkvT = np.ascontiguousarray(np.concatenate([Wq, Wk, Wv], 0).T.astype(bf16))
        woTc = np.ascontiguousarray(Wo[:, sec].T.astype(bf16))
        xTb = np.ascontiguousarray(x[b].T.astype(bf16))
        in_maps.append(
            {
                "xT": xTb,
                "wqkvT": wqkvT,
                "woT": woTc,
                "cosT": cosT4,
                "sinT": sinT4,
                "trid": tri,
            }
        )
    return in_maps


def kernel(x, cos, sin, Wqkv, Wo, _want_profile=False):
    from concourse.bass_utils import run_bass_kernel_spmd

    x = np.asarray(x, dtype=np.float32)
    cos = np.asarray(cos, dtype=np.float32)
    sin = np.asarray(sin, dtype=np.float32)
    Wqkv = np.asarray(Wqkv, dtype=np.float32)
    Wo = np.asarray(Wo, dtype=np.float32)

    nc = _get_program()
    in_maps = _host_inputs(x, cos, sin, Wqkv, Wo)
    res = run_bass_kernel_spmd(nc, in_maps, list(range(NCORES)), trace=_want_profile)
    out = np.empty((B, T, C), dtype=np.float32)
    for b in range(B):
        acc = (
            res.results[2 * b]["outT"].astype(np.float32)
            + res.results[2 * b + 1]["outT"].astype(np.float32)
        )
        out[b] = acc.T
    if _want_profile:
        return out, res
    return out
